# revision 11
# baseline (speedup 1.0000x reference)
"""AxialAttention (MSA row attention) on 8 Trainium2 NeuronCores.

Sharding: pure data parallel over the MSA row dim r=128 (16 rows/core);
the edge-bias precompute is sharded over the edge i dim (32 rows/core)
in a separate first kernel, gathered on host, and replicated into the
attention kernel.

Attention-kernel design (per core):
  - LayerNorm(x) with ln_g / softmax scale folded into projection
    weights on host; LN computes only (x-mu)*rstd via one ACT op.
  - scores are computed transposed, dotsT[j, i], so the softmax sum
    over j and the attn@v contraction both keep j on partitions.
  - per-head bias[h] (from edges, edge-masked on host with -1e38) is
    added into the QK PSUM accumulation via an identity-weight matmul.
  - the column mask enters as the exp() activation's per-partition
    bias: exp(dots + (mask_j-1)*1e38) -> exactly 0 for masked j.
  - softmax denominators come from ones-weight matmuls accumulated in
    PSUM (replicated over each head's 32 partitions).
  - rows with mask_i=0 must produce uniform attention over all j
    (reference semantics); a final copy_predicated overwrites those
    output columns with (mean_j v) * sigmoid(g).
  - head slots use a 3-heads-per-128-block layout at partition offsets
    {0, 32, 64} (hardware requires operand/output partition base in
    {0, 32, 64}), so SLOTS = 3*128 = 384 with zero padding.
"""

import sys
import numpy as np

sys.path.insert(0, "/opt/trn_rl_repo")

import concourse.bacc as bacc
import concourse.tile as tile
import concourse.bass as bass
from concourse import mybir
from concourse import bass_utils
from concourse.masks import make_identity

F32 = mybir.dt.float32
AF = mybir.ActivationFunctionType
MUL = mybir.AluOpType.mult

NC = 8          # cores
B, R, W, DN = 1, 128, 256, 256
DE, H, DH = 128, 8, 32
RPC = R // NC   # rows per core = 16
IPC = W // NC   # edge i-rows per core = 32
NEG = -1.0e38
EPS = 1e-5

NB = 3                      # head blocks (3/3/2 heads)
SLOTS = NB * 128            # 384
HB_ROWS = [96, 96, 64]      # used partitions per block


def _head_slot(h):
    return (h // 3) * 128 + 32 * (h % 3)


def _expand_cols(Wm):
    D = Wm.shape[0]
    out = np.zeros((D, SLOTS), Wm.dtype)
    for h in range(H):
        out[:, _head_slot(h):_head_slot(h) + DH] = Wm[:, h * DH:(h + 1) * DH]
    return out


def _expand_rows(Wm):
    D = Wm.shape[1]
    out = np.zeros((SLOTS, D), Wm.dtype)
    for h in range(H):
        out[_head_slot(h):_head_slot(h) + DH, :] = Wm[h * DH:(h + 1) * DH, :]
    return out


def _ln_smalls(nc, pool, mv, eps_sb):
    """mean/var [P,2] -> (rstd, -mu*rstd) tiles [P,1]."""
    P = mv.shape[0]
    sd = pool.tile([P, 1], F32, tag="sd")
    nc.scalar.activation(sd, mv[:, 1:2], AF.Sqrt, bias=eps_sb[:])
    rstd = pool.tile([P, 1], F32, tag="rs")
    nc.vector.reciprocal(rstd, sd)
    nmr = pool.tile([P, 1], F32, tag="nm")
    nc.vector.scalar_tensor_tensor(out=nmr, in0=mv[:, 0:1], scalar=-1.0,
                                   in1=rstd, op0=MUL, op1=MUL)
    return rstd, nmr


# ---------------------------------------------------------------- kernel 1
def _build_bias_nc():
    """Per core: edges slice [IPC*W, DE] -> bias part [H, IPC*W]."""
    nc = bacc.Bacc("TRN2", target_bir_lowering=False, debug=False,
                   num_devices=NC)
    TOK = IPC * W  # 8192
    e_d = nc.dram_tensor("e", [TOK, DE], F32, kind="ExternalInput").ap()
    we_d = nc.dram_tensor("we", [DE, H], F32, kind="ExternalInput").ap()
    o_d = nc.dram_tensor("o", [H, TOK], F32, kind="ExternalOutput").ap()

    P = 128
    ntiles = TOK // P  # 64

    with tile.TileContext(nc) as tc:
        with tc.tile_pool(name="cst", bufs=1) as cst, \
             tc.tile_pool(name="work", bufs=4) as work, \
             tc.tile_pool(name="tp", bufs=4) as tp, \
             tc.tile_pool(name="pst", bufs=4, space="PSUM") as pst, \
             tc.tile_pool(name="psb", bufs=2, space="PSUM") as psb:
            ident = cst.tile([P, P], F32)
            make_identity(nc, ident[:])
            we_sb = cst.tile([DE, H], F32)
            nc.sync.dma_start(out=we_sb, in_=we_d)
            eps_sb = cst.tile([P, 1], F32)
            nc.vector.memset(eps_sb, EPS)

            for g in range(ntiles // 4):  # groups of 4 tiles -> [8, 512] out
                ob = psb.tile([H, 4 * P], F32, tag="ob")
                for tsub in range(4):
                    t = g * 4 + tsub
                    et = work.tile([P, DE], F32, tag="et")
                    nc.sync.dma_start(out=et, in_=e_d[t * P:(t + 1) * P, :])
                    stats = work.tile([P, 6], F32, tag="st")
                    nc.vector.bn_stats(out=stats, in_=et)
                    mv = work.tile([P, 2], F32, tag="mv")
                    nc.vector.bn_aggr(out=mv, in_=stats)
                    rstd, nmr = _ln_smalls(nc, work, mv, eps_sb)
                    en = work.tile([P, DE], F32, tag="en")
                    nc.scalar.activation(en, et, AF.Identity,
                                         bias=nmr[:], scale=rstd[:])
                    pt = pst.tile([DE, P], F32, tag="pt")
                    nc.tensor.transpose(pt[:], en[:], ident[:])
                    enT = tp.tile([DE, P], F32, tag="enT")
                    nc.vector.tensor_copy(out=enT, in_=pt)
                    nc.tensor.matmul(ob[:, tsub * P:(tsub + 1) * P],
                                     we_sb[:], enT[:], start=True, stop=True)
                ost = work.tile([H, 4 * P], F32, tag="ost")
                nc.vector.tensor_copy(out=ost, in_=ob)
                nc.sync.dma_start(out=o_d[:, g * 4 * P:(g + 1) * 4 * P],
                                  in_=ost)
    nc.compile()
    return nc


# ---------------------------------------------------------------- kernel 2
def _build_attn_nc():
    nc = bacc.Bacc("TRN2", target_bir_lowering=False, debug=False,
                   num_devices=NC)
    P = 128
    TOK = RPC * W          # 4096 tokens per core
    CH = 512               # tokens per chunk (2 rows)
    NCH = TOK // CH        # 8 chunks
    ROWS_PER_CH = CH // W  # 2

    x_d = nc.dram_tensor("x", [TOK, DN], F32, kind="ExternalInput").ap()
    wq_d = nc.dram_tensor("wq", [DN, SLOTS], F32, kind="ExternalInput").ap()
    wk_d = nc.dram_tensor("wk", [DN, SLOTS], F32, kind="ExternalInput").ap()
    wv_d = nc.dram_tensor("wv", [DN, SLOTS], F32, kind="ExternalInput").ap()
    wg_d = nc.dram_tensor("wg", [DN, SLOTS], F32, kind="ExternalInput").ap()
    wo_d = nc.dram_tensor("wo", [SLOTS, DN], F32, kind="ExternalInput").ap()
    bg_d = nc.dram_tensor("bg", [P, NB], F32, kind="ExternalInput").ap()
    bo_d = nc.dram_tensor("bo", [1, DN], F32, kind="ExternalInput").ap()
    bt_d = nc.dram_tensor("bt", [P, H, 2, W], F32, kind="ExternalInput").ap()
    ngj_d = nc.dram_tensor("ngj", [P, RPC * 2], F32,
                           kind="ExternalInput").ap()
    invm_d = nc.dram_tensor("invm", [RPC, W], mybir.dt.uint8,
                            kind="ExternalInput").ap()
    o_d = nc.dram_tensor("o", [TOK, DN], F32, kind="ExternalOutput").ap()

    with tile.TileContext(nc) as tc:
        from contextlib import ExitStack
        with ExitStack() as ctx:
            cst = ctx.enter_context(tc.tile_pool(name="cst", bufs=1))
            lnw = ctx.enter_context(tc.tile_pool(name="lnw", bufs=4))
            chw = ctx.enter_context(tc.tile_pool(name="chw", bufs=2))
            expp = ctx.enter_context(tc.tile_pool(name="expp", bufs=4))
            rowp = ctx.enter_context(tc.tile_pool(name="rowp", bufs=2))
            ps_sc = ctx.enter_context(
                tc.tile_pool(name="ps_sc", bufs=5, space="PSUM"))
            ps_av = ctx.enter_context(
                tc.tile_pool(name="ps_av", bufs=1, space="PSUM"))
            ps_sm = ctx.enter_context(
                tc.tile_pool(name="ps_sm", bufs=1, space="PSUM"))

            ident = cst.tile([P, P], F32)
            make_identity(nc, ident[:])
            ones_blk = cst.tile([P, 32], F32)  # ones: lhsT for S matmuls
            nc.vector.memset(ones_blk, 1.0)
            ones_row = cst.tile([1, P], F32)   # lhsT for rank-1 bo add
            nc.vector.memset(ones_row, 1.0)
            eps_sb = cst.tile([P, 1], F32)
            nc.vector.memset(eps_sb, EPS)

            def load_w(d, shape, nm):
                t = cst.tile(shape, F32, tag=nm, name=nm)
                nc.sync.dma_start(out=t, in_=d)
                return t

            wq = [load_w(wq_d[kt * P:(kt + 1) * P, :], [P, SLOTS], f"wq{kt}")
                  for kt in range(2)]
            wk = [load_w(wk_d[kt * P:(kt + 1) * P, :], [P, SLOTS], f"wk{kt}")
                  for kt in range(2)]
            wv = [load_w(wv_d[kt * P:(kt + 1) * P, :], [P, SLOTS], f"wv{kt}")
                  for kt in range(2)]
            wg = [load_w(wg_d[kt * P:(kt + 1) * P, :], [P, SLOTS], f"wg{kt}")
                  for kt in range(2)]
            wo = [load_w(wo_d[b * P:b * P + HB_ROWS[b], :],
                         [HB_ROWS[b], DN], f"wo{b}") for b in range(NB)]
            bg = load_w(bg_d, [P, NB], "bgt")
            bo = load_w(bo_d, [1, DN], "bot")
            bt_sb = load_w(bt_d, [P, H, 2, W], "btt")
            ngj = load_w(ngj_d, [P, RPC * 2], "ngjt")

            for ch in range(NCH):
                tok0 = ch * CH
                # ---- LN + transpose: xnT [2][P, CH]
                xnT_ps = [ps_sc.tile([P, CH], F32, tag="sc",
                                     name=f"xnT_ps{ch}_{kt}")
                          for kt in range(2)]
                for ts in range(CH // P):
                    xt = lnw.tile([P, DN], F32, tag="xt")
                    nc.sync.dma_start(
                        out=xt, in_=x_d[tok0 + ts * P:tok0 + (ts + 1) * P, :])
                    stats = lnw.tile([P, 6], F32, tag="st")
                    nc.vector.bn_stats(out=stats, in_=xt)
                    mv = lnw.tile([P, 2], F32, tag="mv")
                    nc.vector.bn_aggr(out=mv, in_=stats)
                    rstd, nmr = _ln_smalls(nc, lnw, mv, eps_sb)
                    xn = lnw.tile([P, DN], F32, tag="xn")
                    nc.scalar.activation(xn, xt, AF.Identity,
                                         bias=nmr[:], scale=rstd[:])
                    for kt in range(2):
                        nc.tensor.transpose(
                            xnT_ps[kt][:, ts * P:(ts + 1) * P],
                            xn[:, kt * P:(kt + 1) * P], ident[:])
                xnT = [chw.tile([P, CH], F32, tag=f"xnT{kt}",
                                name=f"xnT{ch}_{kt}")
                       for kt in range(2)]
                for kt in range(2):
                    nc.any.tensor_copy(out=xnT[kt], in_=xnT_ps[kt])

                # ---- projections
                def proj_block(ws, b):
                    pp = ps_sc.tile([P, CH], F32, tag="sc")
                    for kt in range(2):
                        nc.tensor.matmul(
                            pp[:], ws[kt][:, b * P:(b + 1) * P],
                            xnT[kt][:], start=(kt == 0), stop=(kt == 1))
                    return pp

                q_sb, k_sb, sig_sb = [], [], []
                for b in range(NB):
                    pp = proj_block(wq, b)
                    t = chw.tile([P, CH], F32, tag=f"q{b}")
                    nc.any.tensor_copy(out=t, in_=pp)
                    q_sb.append(t)
                for b in range(NB):
                    pp = proj_block(wk, b)
                    t = chw.tile([P, CH], F32, tag=f"k{b}")
                    nc.any.tensor_copy(out=t, in_=pp)
                    k_sb.append(t)
                for b in range(NB):
                    pp = proj_block(wg, b)
                    t = chw.tile([P, CH], F32, tag=f"sig{b}")
                    nc.scalar.activation(t, pp, AF.Sigmoid,
                                         bias=bg[:, b:b + 1])
                    sig_sb.append(t)
                v_sb = []
                for tb in range(CH // P):
                    pp = ps_sc.tile([P, SLOTS], F32, tag="sc")
                    for kt in range(2):
                        nc.tensor.matmul(
                            pp[:], xnT[kt][:, tb * P:(tb + 1) * P],
                            wv[kt][:], start=(kt == 0), stop=(kt == 1))
                    t = chw.tile([P, SLOTS], F32, tag=f"v{tb}")
                    nc.any.tensor_copy(out=t, in_=pp)
                    v_sb.append(t)

                # ---- per-row attention (one head-block at a time)
                for rl in range(ROWS_PER_CH):
                    r = ch * ROWS_PER_CH + rl
                    i0 = rl * W
                    invm_b = rowp.tile([P, W], mybir.dt.uint8, tag="invm_b")
                    nc.sync.dma_start(
                        out=invm_b,
                        in_=bass.AP(tensor=invm_d.tensor, offset=r * W,
                                    ap=[[0, P], [1, W]]))
                    vbar = ps_sm.tile([P, 4], F32, tag="vbar")

                    oTg = []
                    for b in range(NB):
                        hbr = HB_ROWS[b]
                        nheads = hbr // 32
                        sbig = ps_sm.tile([P, W], F32, tag="sbig")
                        av = ps_av.tile([P, W], F32, tag="av")
                        expT = [[None] * 2 for _ in range(nheads)]
                        for jt in range(2):
                            for u in range(nheads):
                                h = 3 * b + u
                                ho = 32 * u
                                dots = ps_sc.tile([P, W], F32, tag="sc")
                                nc.tensor.matmul(
                                    dots[:], ident[:], bt_sb[:, h, jt, :],
                                    start=True, stop=False)
                                nc.tensor.matmul(
                                    dots[:],
                                    k_sb[b][ho:ho + DH,
                                            i0 + jt * P:i0 + (jt + 1) * P],
                                    q_sb[b][ho:ho + DH, i0:i0 + W],
                                    start=False, stop=True)
                                et = expp.tile([P, W], F32, tag="expT")
                                nc.scalar.activation(
                                    et, dots, AF.Exp,
                                    bias=ngj[:, r * 2 + jt:r * 2 + jt + 1])
                                expT[u][jt] = et
                                nc.tensor.matmul(
                                    sbig[ho:ho + 32, :], ones_blk[:], et[:],
                                    start=(jt == 0), stop=(jt == 1))
                                nc.tensor.matmul(
                                    av[ho:ho + DH, :],
                                    v_sb[2 * rl + jt][:, b * P + ho:
                                                      b * P + ho + DH],
                                    et[:], start=(jt == 0), stop=(jt == 1))
                            nc.tensor.matmul(
                                vbar[:, b:b + 1],
                                v_sb[2 * rl + jt][:, b * P:(b + 1) * P],
                                ones_blk[:, 0:1], start=(jt == 0),
                                stop=(jt == 1))

                        rbig = rowp.tile([P, W], F32, tag="rbig")
                        nc.vector.reciprocal_approx_fast(
                            rbig[0:hbr], sbig[0:hbr])
                        t1 = rowp.tile([P, W], F32, tag="t1")
                        nc.vector.scalar_tensor_tensor(
                            out=t1[0:hbr], in0=av[0:hbr], scalar=1.0,
                            in1=rbig[0:hbr], op0=MUL, op1=MUL)
                        og = rowp.tile([P, W], F32, tag=f"og{b}")
                        nc.vector.tensor_tensor(
                            out=og[0:hbr], in0=t1[0:hbr],
                            in1=sig_sb[b][0:hbr, i0:i0 + W], op=MUL)
                        vbs = rowp.tile([P, W], F32, tag="vbs")
                        nc.vector.tensor_scalar(
                            out=vbs[0:hbr], in0=sig_sb[b][0:hbr, i0:i0 + W],
                            scalar1=vbar[0:hbr, b:b + 1], scalar2=1.0 / W,
                            op0=MUL, op1=MUL)
                        nc.vector.copy_predicated(
                            out=og[0:hbr], mask=invm_b[0:hbr], data=vbs[0:hbr])
                        oTg.append(og)

                    for ts in range(2):
                        op = ps_sc.tile([P, DN], F32, tag="sc")
                        nc.tensor.matmul(op[:], ones_row[:], bo[:],
                                         start=True, stop=False)
                        for b in range(NB):
                            hbr = HB_ROWS[b]
                            nc.tensor.matmul(
                                op[:], oTg[b][0:hbr, ts * P:(ts + 1) * P],
                                wo[b][:], start=False, stop=(b == NB - 1))
                        ot = rowp.tile([P, DN], F32, tag="ot")
                        nc.any.tensor_copy(out=ot, in_=op)
                        nc.sync.dma_start(
                            out=o_d[tok0 + i0 + ts * P:
                                    tok0 + i0 + (ts + 1) * P, :],
                            in_=ot)
    nc.compile()
    return nc


_NC_CACHE = {}
TRACE = False


def _get_nc(name):
    if name not in _NC_CACHE:
        _NC_CACHE[name] = (_build_bias_nc if name == "bias"
                           else _build_attn_nc)()
    return _NC_CACHE[name]


def build_attn_in_maps(inputs):
    return _prep(**inputs)[1]


def _prep(x, edges, mask, edge_mask, ln_g, ln_b, lne_g, lne_b,
          W_edge, Wq, Wkv, Wg, bg, Wo, bo):
    f32 = np.float32
    x = np.asarray(x, f32)
    edges = np.asarray(edges, f32)
    mask_b = np.asarray(mask).astype(bool)
    edge_mask_b = np.asarray(edge_mask).astype(bool)
    ln_g = np.asarray(ln_g, f32); ln_b = np.asarray(ln_b, f32)
    lne_g = np.asarray(lne_g, f32); lne_b = np.asarray(lne_b, f32)
    W_edge = np.asarray(W_edge, f32)
    Wq = np.asarray(Wq, f32); Wkv = np.asarray(Wkv, f32)
    Wg = np.asarray(Wg, f32); bg = np.asarray(bg, f32)
    Wo = np.asarray(Wo, f32); bo = np.asarray(bo, f32)

    # ---------------- kernel 1: bias from edges
    nc1 = _get_nc("bias")
    we = (lne_g[:, None] * W_edge).astype(f32)
    e_flat = edges.reshape(W, W, DE)
    in_maps1 = []
    for c in range(NC):
        in_maps1.append({
            "e": np.ascontiguousarray(
                e_flat[c * IPC:(c + 1) * IPC].reshape(IPC * W, DE)),
            "we": we,
        })
    res1 = bass_utils.run_bass_kernel_spmd(nc1, in_maps1,
                                           core_ids=list(range(NC)),
                                           trace=TRACE)
    if TRACE:
        print("bias kernel exec_time_ns:", res1.exec_time_ns)
    bias = np.concatenate(
        [res1.results[c]["o"].reshape(H, IPC, W) for c in range(NC)],
        axis=1)  # [H, i, j]
    bias = bias + (lne_b @ W_edge)[:, None, None]
    bias = np.where(edge_mask_b[0][None], bias, NEG).astype(f32)
    biasT = np.ascontiguousarray(bias.transpose(0, 2, 1))  # [H, j, i]
    bt = np.ascontiguousarray(
        biasT.reshape(H, 2, 128, W).transpose(2, 0, 1, 3))

    # ---------------- kernel 2: attention
    nc2 = _get_nc("attn")
    scale = DH ** -0.5
    Wk_, Wv_ = Wkv[:, :H * DH], Wkv[:, H * DH:]
    gq = _expand_cols((ln_g[:, None] * Wq * scale).astype(f32))
    gk = _expand_cols((ln_g[:, None] * Wk_).astype(f32))
    gv = _expand_cols((ln_g[:, None] * Wv_).astype(f32))
    gg = _expand_cols((ln_g[:, None] * Wg).astype(f32))
    # the reference applies LN bias ln_b before projections; fold it in.
    # q gets +ln_b@Wq etc.  For q/k this shifts dots identically across
    # j only through k (rank-1 in j) -- NOT dropped; instead we require
    # ln_b == 0 (true for this problem's inputs) and assert.
    assert np.allclose(ln_b, 0.0), "ln_b folding not implemented"
    bgx = np.zeros((128, NB), f32)
    for h in range(H):
        bgx[32 * (h % 3):32 * (h % 3) + DH, h // 3] = bg[h * DH:(h + 1) * DH]
    woe = _expand_rows(Wo.astype(f32))

    maskf = mask_b[0].astype(f32)  # [R, W]
    x_flat = x.reshape(R, W, DN)
    in_maps2 = []
    for c in range(NC):
        mrows = maskf[c * RPC:(c + 1) * RPC]  # [RPC, W]
        ngj = (mrows.reshape(RPC, 2, 128) - 1.0) * 1e38  # [r, jt, p]
        ngj = np.ascontiguousarray(
            ngj.transpose(2, 0, 1).reshape(128, RPC * 2))
        in_maps2.append({
            "x": np.ascontiguousarray(
                x_flat[c * RPC:(c + 1) * RPC].reshape(RPC * W, DN)),
            "wq": gq, "wk": gk, "wv": gv, "wg": gg, "wo": woe,
            "bg": bgx, "bo": bo.reshape(1, DN).astype(f32),
            "bt": bt, "ngj": ngj.astype(f32),
            "invm": (1.0 - mrows).astype(np.uint8),
        })
    return nc2, in_maps2


def kernel(**inputs):
    nc2, in_maps2 = _prep(**inputs)
    res2 = bass_utils.run_bass_kernel_spmd(nc2, in_maps2,
                                           core_ids=list(range(NC)),
                                           trace=TRACE)
    if TRACE:
        print("attn kernel exec_time_ns:", res2.exec_time_ns)
    out = np.concatenate(
        [res2.results[c]["o"].reshape(RPC, W, DN) for c in range(NC)],
        axis=0)
    return out.reshape(B, R, W, DN).astype(np.float32)


# revision 12
# speedup vs baseline: 227.5671x; 227.5671x over previous
"""AxialAttention (MSA row attention) on 8 Trainium2 NeuronCores.

Sharding: pure data parallel over the MSA row dim r=128 (16 rows/core);
the edge-bias precompute is sharded over the edge i dim (32 rows/core)
in a separate first kernel, gathered on host, and replicated into the
attention kernel.

Attention-kernel design (per core):
  - LayerNorm(x) with ln_g / softmax scale folded into projection
    weights on host; LN computes only (x-mu)*rstd via one ACT op.
  - scores are computed transposed, dotsT[j, i], so the softmax sum
    over j and the attn@v contraction both keep j on partitions.
  - per-head bias[h] (from edges, edge-masked on host with -1e38) is
    added into the QK PSUM accumulation via an identity-weight matmul.
  - the column mask enters as the exp() activation's per-partition
    bias: exp(dots + (mask_j-1)*1e38) -> exactly 0 for masked j.
  - softmax denominators come from ones-weight matmuls accumulated in
    PSUM (replicated over each head's 32 partitions).
  - rows with mask_i=0 must produce uniform attention over all j
    (reference semantics); a final copy_predicated overwrites those
    output columns with (mean_j v) * sigmoid(g).
  - head slots use a 3-heads-per-128-block layout at partition offsets
    {0, 32, 64} (hardware requires operand/output partition base in
    {0, 32, 64}), so SLOTS = 3*128 = 384 with zero padding.
"""

import sys
import numpy as np

sys.path.insert(0, "/opt/trn_rl_repo")

import concourse.bacc as bacc
import concourse.tile as tile
import concourse.bass as bass
from concourse import mybir
from concourse import bass_utils
from concourse.masks import make_identity

F32 = mybir.dt.float32
AF = mybir.ActivationFunctionType
MUL = mybir.AluOpType.mult

NC = 8          # cores
B, R, W, DN = 1, 128, 256, 256
DE, H, DH = 128, 8, 32
RPC = R // NC   # rows per core = 16
IPC = W // NC   # edge i-rows per core = 32
NEG = -1.0e38
EPS = 1e-5

NB = 3                      # head blocks (3/3/2 heads)
SLOTS = NB * 128            # 384
HB_ROWS = [96, 96, 64]      # used partitions per block


def _head_slot(h):
    return (h // 3) * 128 + 32 * (h % 3)


def _expand_cols(Wm):
    D = Wm.shape[0]
    out = np.zeros((D, SLOTS), Wm.dtype)
    for h in range(H):
        out[:, _head_slot(h):_head_slot(h) + DH] = Wm[:, h * DH:(h + 1) * DH]
    return out


def _expand_rows(Wm):
    D = Wm.shape[1]
    out = np.zeros((SLOTS, D), Wm.dtype)
    for h in range(H):
        out[_head_slot(h):_head_slot(h) + DH, :] = Wm[h * DH:(h + 1) * DH, :]
    return out


def _ln_smalls(nc, pool, mv, eps_sb):
    """mean/var [P,2] -> (rstd, -mu*rstd) tiles [P,1]."""
    P = mv.shape[0]
    sd = pool.tile([P, 1], F32, tag="sd")
    nc.scalar.activation(sd, mv[:, 1:2], AF.Sqrt, bias=eps_sb[:])
    rstd = pool.tile([P, 1], F32, tag="rs")
    nc.vector.reciprocal(rstd, sd)
    nmr = pool.tile([P, 1], F32, tag="nm")
    nc.vector.scalar_tensor_tensor(out=nmr, in0=mv[:, 0:1], scalar=-1.0,
                                   in1=rstd, op0=MUL, op1=MUL)
    return rstd, nmr


# ---------------------------------------------------------------- kernel 1
def _build_bias_nc():
    """Per core: edges slice [IPC*W, DE] -> bias part [H, IPC*W]."""
    nc = bacc.Bacc("TRN2", target_bir_lowering=False, debug=False,
                   num_devices=NC)
    TOK = IPC * W  # 8192
    e_d = nc.dram_tensor("e", [TOK, DE], F32, kind="ExternalInput").ap()
    we_d = nc.dram_tensor("we", [DE, H], F32, kind="ExternalInput").ap()
    o_d = nc.dram_tensor("o", [H, TOK], F32, kind="ExternalOutput").ap()

    P = 128
    ntiles = TOK // P  # 64

    with tile.TileContext(nc) as tc:
        with tc.tile_pool(name="cst", bufs=1) as cst, \
             tc.tile_pool(name="work", bufs=4) as work, \
             tc.tile_pool(name="tp", bufs=4) as tp, \
             tc.tile_pool(name="pst", bufs=4, space="PSUM") as pst, \
             tc.tile_pool(name="psb", bufs=2, space="PSUM") as psb:
            ident = cst.tile([P, P], F32)
            make_identity(nc, ident[:])
            we_sb = cst.tile([DE, H], F32)
            nc.sync.dma_start(out=we_sb, in_=we_d)
            eps_sb = cst.tile([P, 1], F32)
            nc.vector.memset(eps_sb, EPS)

            for g in [gg for _ in range(REPEAT)
                      for gg in range(ntiles // 4)]:
                ob = psb.tile([H, 4 * P], F32, tag="ob")
                for tsub in range(4):
                    t = g * 4 + tsub
                    et = work.tile([P, DE], F32, tag="et")
                    nc.sync.dma_start(out=et, in_=e_d[t * P:(t + 1) * P, :])
                    stats = work.tile([P, 6], F32, tag="st")
                    nc.vector.bn_stats(out=stats, in_=et)
                    mv = work.tile([P, 2], F32, tag="mv")
                    nc.vector.bn_aggr(out=mv, in_=stats)
                    rstd, nmr = _ln_smalls(nc, work, mv, eps_sb)
                    en = work.tile([P, DE], F32, tag="en")
                    nc.scalar.activation(en, et, AF.Identity,
                                         bias=nmr[:], scale=rstd[:])
                    pt = pst.tile([DE, P], F32, tag="pt")
                    nc.tensor.transpose(pt[:], en[:], ident[:])
                    enT = tp.tile([DE, P], F32, tag="enT")
                    nc.vector.tensor_copy(out=enT, in_=pt)
                    nc.tensor.matmul(ob[:, tsub * P:(tsub + 1) * P],
                                     we_sb[:], enT[:], start=True, stop=True)
                ost = work.tile([H, 4 * P], F32, tag="ost")
                nc.vector.tensor_copy(out=ost, in_=ob)
                nc.sync.dma_start(out=o_d[:, g * 4 * P:(g + 1) * 4 * P],
                                  in_=ost)
    nc.compile()
    return nc


# ---------------------------------------------------------------- kernel 2
def _build_attn_nc():
    nc = bacc.Bacc("TRN2", target_bir_lowering=False, debug=False,
                   num_devices=NC)
    P = 128
    TOK = RPC * W          # 4096 tokens per core
    CH = 512               # tokens per chunk (2 rows)
    NCH = TOK // CH        # 8 chunks
    ROWS_PER_CH = CH // W  # 2

    x_d = nc.dram_tensor("x", [TOK, DN], F32, kind="ExternalInput").ap()
    wq_d = nc.dram_tensor("wq", [DN, SLOTS], F32, kind="ExternalInput").ap()
    wk_d = nc.dram_tensor("wk", [DN, SLOTS], F32, kind="ExternalInput").ap()
    wv_d = nc.dram_tensor("wv", [DN, SLOTS], F32, kind="ExternalInput").ap()
    wg_d = nc.dram_tensor("wg", [DN, SLOTS], F32, kind="ExternalInput").ap()
    wo_d = nc.dram_tensor("wo", [SLOTS, DN], F32, kind="ExternalInput").ap()
    bg_d = nc.dram_tensor("bg", [P, NB], F32, kind="ExternalInput").ap()
    bo_d = nc.dram_tensor("bo", [1, DN], F32, kind="ExternalInput").ap()
    bt_d = nc.dram_tensor("bt", [P, H, 2, W], F32, kind="ExternalInput").ap()
    ngj_d = nc.dram_tensor("ngj", [P, RPC * 2], F32,
                           kind="ExternalInput").ap()
    invm_d = nc.dram_tensor("invm", [RPC, W], mybir.dt.uint8,
                            kind="ExternalInput").ap()
    o_d = nc.dram_tensor("o", [TOK, DN], F32, kind="ExternalOutput").ap()

    with tile.TileContext(nc) as tc:
        from contextlib import ExitStack
        with ExitStack() as ctx:
            cst = ctx.enter_context(tc.tile_pool(name="cst", bufs=1))
            lnw = ctx.enter_context(tc.tile_pool(name="lnw", bufs=4))
            chw = ctx.enter_context(tc.tile_pool(name="chw", bufs=2))
            expp = ctx.enter_context(tc.tile_pool(name="expp", bufs=4))
            rowp = ctx.enter_context(tc.tile_pool(name="rowp", bufs=2))
            ps_sc = ctx.enter_context(
                tc.tile_pool(name="ps_sc", bufs=5, space="PSUM"))
            ps_av = ctx.enter_context(
                tc.tile_pool(name="ps_av", bufs=1, space="PSUM"))
            ps_sm = ctx.enter_context(
                tc.tile_pool(name="ps_sm", bufs=1, space="PSUM"))

            ident = cst.tile([P, P], F32)
            make_identity(nc, ident[:])
            ones_blk = cst.tile([P, 32], F32)  # ones: lhsT for S matmuls
            nc.vector.memset(ones_blk, 1.0)
            ones_row = cst.tile([1, P], F32)   # lhsT for rank-1 bo add
            nc.vector.memset(ones_row, 1.0)
            eps_sb = cst.tile([P, 1], F32)
            nc.vector.memset(eps_sb, EPS)

            def load_w(d, shape, nm):
                t = cst.tile(shape, F32, tag=nm, name=nm)
                nc.sync.dma_start(out=t, in_=d)
                return t

            wq = [load_w(wq_d[kt * P:(kt + 1) * P, :], [P, SLOTS], f"wq{kt}")
                  for kt in range(2)]
            wk = [load_w(wk_d[kt * P:(kt + 1) * P, :], [P, SLOTS], f"wk{kt}")
                  for kt in range(2)]
            wv = [load_w(wv_d[kt * P:(kt + 1) * P, :], [P, SLOTS], f"wv{kt}")
                  for kt in range(2)]
            wg = [load_w(wg_d[kt * P:(kt + 1) * P, :], [P, SLOTS], f"wg{kt}")
                  for kt in range(2)]
            wo = [load_w(wo_d[b * P:b * P + HB_ROWS[b], :],
                         [HB_ROWS[b], DN], f"wo{b}") for b in range(NB)]
            bg = load_w(bg_d, [P, NB], "bgt")
            bo = load_w(bo_d, [1, DN], "bot")
            bt_sb = load_w(bt_d, [P, H, 2, W], "btt")
            ngj = load_w(ngj_d, [P, RPC * 2], "ngjt")

            for ch in [cc for _ in range(REPEAT) for cc in range(NCH)]:
                tok0 = ch * CH
                # ---- LN + transpose: xnT [2][P, CH]
                xnT_ps = [ps_sc.tile([P, CH], F32, tag="sc",
                                     name=f"xnT_ps{ch}_{kt}")
                          for kt in range(2)]
                for ts in range(CH // P):
                    xt = lnw.tile([P, DN], F32, tag="xt")
                    nc.sync.dma_start(
                        out=xt, in_=x_d[tok0 + ts * P:tok0 + (ts + 1) * P, :])
                    stats = lnw.tile([P, 6], F32, tag="st")
                    nc.vector.bn_stats(out=stats, in_=xt)
                    mv = lnw.tile([P, 2], F32, tag="mv")
                    nc.vector.bn_aggr(out=mv, in_=stats)
                    rstd, nmr = _ln_smalls(nc, lnw, mv, eps_sb)
                    xn = lnw.tile([P, DN], F32, tag="xn")
                    nc.scalar.activation(xn, xt, AF.Identity,
                                         bias=nmr[:], scale=rstd[:])
                    for kt in range(2):
                        nc.tensor.transpose(
                            xnT_ps[kt][:, ts * P:(ts + 1) * P],
                            xn[:, kt * P:(kt + 1) * P], ident[:])
                xnT = [chw.tile([P, CH], F32, tag=f"xnT{kt}",
                                name=f"xnT{ch}_{kt}")
                       for kt in range(2)]
                for kt in range(2):
                    nc.any.tensor_copy(out=xnT[kt], in_=xnT_ps[kt])

                # ---- projections
                def proj_block(ws, b):
                    pp = ps_sc.tile([P, CH], F32, tag="sc")
                    for kt in range(2):
                        nc.tensor.matmul(
                            pp[:], ws[kt][:, b * P:(b + 1) * P],
                            xnT[kt][:], start=(kt == 0), stop=(kt == 1))
                    return pp

                q_sb, k_sb, sig_sb = [], [], []
                for b in range(NB):
                    pp = proj_block(wq, b)
                    t = chw.tile([P, CH], F32, tag=f"q{b}")
                    nc.any.tensor_copy(out=t, in_=pp)
                    q_sb.append(t)
                for b in range(NB):
                    pp = proj_block(wk, b)
                    t = chw.tile([P, CH], F32, tag=f"k{b}")
                    nc.any.tensor_copy(out=t, in_=pp)
                    k_sb.append(t)
                for b in range(NB):
                    pp = proj_block(wg, b)
                    t = chw.tile([P, CH], F32, tag=f"sig{b}")
                    nc.scalar.activation(t, pp, AF.Sigmoid,
                                         bias=bg[:, b:b + 1])
                    sig_sb.append(t)
                v_sb = []
                for tb in range(CH // P):
                    pp = ps_sc.tile([P, SLOTS], F32, tag="sc")
                    for kt in range(2):
                        nc.tensor.matmul(
                            pp[:], xnT[kt][:, tb * P:(tb + 1) * P],
                            wv[kt][:], start=(kt == 0), stop=(kt == 1))
                    t = chw.tile([P, SLOTS], F32, tag=f"v{tb}")
                    nc.any.tensor_copy(out=t, in_=pp)
                    v_sb.append(t)

                # ---- per-row attention (one head-block at a time)
                for rl in range(ROWS_PER_CH):
                    r = ch * ROWS_PER_CH + rl
                    i0 = rl * W
                    invm_b = rowp.tile([P, W], mybir.dt.uint8, tag="invm_b")
                    nc.sync.dma_start(
                        out=invm_b,
                        in_=bass.AP(tensor=invm_d.tensor, offset=r * W,
                                    ap=[[0, P], [1, W]]))
                    vbar = ps_sm.tile([P, 4], F32, tag="vbar")

                    oTg = []
                    for b in range(NB):
                        hbr = HB_ROWS[b]
                        nheads = hbr // 32
                        sbig = ps_sm.tile([P, W], F32, tag="sbig")
                        av = ps_av.tile([P, W], F32, tag="av")
                        expT = [[None] * 2 for _ in range(nheads)]
                        for jt in range(2):
                            for u in range(nheads):
                                h = 3 * b + u
                                ho = 32 * u
                                dots = ps_sc.tile([P, W], F32, tag="sc")
                                nc.tensor.matmul(
                                    dots[:], ident[:], bt_sb[:, h, jt, :],
                                    start=True, stop=False)
                                nc.tensor.matmul(
                                    dots[:],
                                    k_sb[b][ho:ho + DH,
                                            i0 + jt * P:i0 + (jt + 1) * P],
                                    q_sb[b][ho:ho + DH, i0:i0 + W],
                                    start=False, stop=True)
                                et = expp.tile([P, W], F32, tag="expT")
                                nc.scalar.activation(
                                    et, dots, AF.Exp,
                                    bias=ngj[:, r * 2 + jt:r * 2 + jt + 1])
                                expT[u][jt] = et
                                nc.tensor.matmul(
                                    sbig[ho:ho + 32, :], ones_blk[:], et[:],
                                    start=(jt == 0), stop=(jt == 1))
                                nc.tensor.matmul(
                                    av[ho:ho + DH, :],
                                    v_sb[2 * rl + jt][:, b * P + ho:
                                                      b * P + ho + DH],
                                    et[:], start=(jt == 0), stop=(jt == 1))
                            nc.tensor.matmul(
                                vbar[:, b:b + 1],
                                v_sb[2 * rl + jt][:, b * P:(b + 1) * P],
                                ones_blk[:, 0:1], start=(jt == 0),
                                stop=(jt == 1))

                        rbig = rowp.tile([P, W], F32, tag="rbig")
                        nc.vector.reciprocal_approx_fast(
                            rbig[0:hbr], sbig[0:hbr])
                        t1 = rowp.tile([P, W], F32, tag="t1")
                        nc.vector.scalar_tensor_tensor(
                            out=t1[0:hbr], in0=av[0:hbr], scalar=1.0,
                            in1=rbig[0:hbr], op0=MUL, op1=MUL)
                        og = rowp.tile([P, W], F32, tag=f"og{b}")
                        nc.vector.tensor_tensor(
                            out=og[0:hbr], in0=t1[0:hbr],
                            in1=sig_sb[b][0:hbr, i0:i0 + W], op=MUL)
                        vbs = rowp.tile([P, W], F32, tag="vbs")
                        nc.vector.tensor_scalar(
                            out=vbs[0:hbr], in0=sig_sb[b][0:hbr, i0:i0 + W],
                            scalar1=vbar[0:hbr, b:b + 1], scalar2=1.0 / W,
                            op0=MUL, op1=MUL)
                        nc.vector.copy_predicated(
                            out=og[0:hbr], mask=invm_b[0:hbr], data=vbs[0:hbr])
                        oTg.append(og)

                    for ts in range(2):
                        op = ps_sc.tile([P, DN], F32, tag="sc")
                        nc.tensor.matmul(op[:], ones_row[:], bo[:],
                                         start=True, stop=False)
                        for b in range(NB):
                            hbr = HB_ROWS[b]
                            nc.tensor.matmul(
                                op[:], oTg[b][0:hbr, ts * P:(ts + 1) * P],
                                wo[b][:], start=False, stop=(b == NB - 1))
                        ot = rowp.tile([P, DN], F32, tag="ot")
                        nc.any.tensor_copy(out=ot, in_=op)
                        nc.sync.dma_start(
                            out=o_d[tok0 + i0 + ts * P:
                                    tok0 + i0 + (ts + 1) * P, :],
                            in_=ot)
    nc.compile()
    return nc


_NC_CACHE = {}
TRACE = False
REPEAT = 1


def _get_nc(name):
    key = (name, REPEAT)
    if key not in _NC_CACHE:
        _NC_CACHE[key] = (_build_bias_nc if name == "bias"
                          else _build_attn_nc)()
    return _NC_CACHE[key]


def build_attn_in_maps(inputs):
    return _prep(**inputs)[1]


def _prep(x, edges, mask, edge_mask, ln_g, ln_b, lne_g, lne_b,
          W_edge, Wq, Wkv, Wg, bg, Wo, bo):
    f32 = np.float32
    x = np.asarray(x, f32)
    edges = np.asarray(edges, f32)
    mask_b = np.asarray(mask).astype(bool)
    edge_mask_b = np.asarray(edge_mask).astype(bool)
    ln_g = np.asarray(ln_g, f32); ln_b = np.asarray(ln_b, f32)
    lne_g = np.asarray(lne_g, f32); lne_b = np.asarray(lne_b, f32)
    W_edge = np.asarray(W_edge, f32)
    Wq = np.asarray(Wq, f32); Wkv = np.asarray(Wkv, f32)
    Wg = np.asarray(Wg, f32); bg = np.asarray(bg, f32)
    Wo = np.asarray(Wo, f32); bo = np.asarray(bo, f32)

    # ---------------- kernel 1: bias from edges
    nc1 = _get_nc("bias")
    we = (lne_g[:, None] * W_edge).astype(f32)
    e_flat = edges.reshape(W, W, DE)
    in_maps1 = []
    for c in range(NC):
        in_maps1.append({
            "e": np.ascontiguousarray(
                e_flat[c * IPC:(c + 1) * IPC].reshape(IPC * W, DE)),
            "we": we,
        })
    res1 = bass_utils.run_bass_kernel_spmd(nc1, in_maps1,
                                           core_ids=list(range(NC)),
                                           trace=TRACE)
    if TRACE:
        print("bias kernel exec_time_ns:", res1.exec_time_ns)
    bias = np.concatenate(
        [res1.results[c]["o"].reshape(H, IPC, W) for c in range(NC)],
        axis=1)  # [H, i, j]
    bias = bias + (lne_b @ W_edge)[:, None, None]
    bias = np.where(edge_mask_b[0][None], bias, NEG).astype(f32)
    biasT = np.ascontiguousarray(bias.transpose(0, 2, 1))  # [H, j, i]
    bt = np.ascontiguousarray(
        biasT.reshape(H, 2, 128, W).transpose(2, 0, 1, 3))

    # ---------------- kernel 2: attention
    nc2 = _get_nc("attn")
    scale = DH ** -0.5
    Wk_, Wv_ = Wkv[:, :H * DH], Wkv[:, H * DH:]
    gq = _expand_cols((ln_g[:, None] * Wq * scale).astype(f32))
    gk = _expand_cols((ln_g[:, None] * Wk_).astype(f32))
    gv = _expand_cols((ln_g[:, None] * Wv_).astype(f32))
    gg = _expand_cols((ln_g[:, None] * Wg).astype(f32))
    # the reference applies LN bias ln_b before projections; fold it in.
    # q gets +ln_b@Wq etc.  For q/k this shifts dots identically across
    # j only through k (rank-1 in j) -- NOT dropped; instead we require
    # ln_b == 0 (true for this problem's inputs) and assert.
    assert np.allclose(ln_b, 0.0), "ln_b folding not implemented"
    bgx = np.zeros((128, NB), f32)
    for h in range(H):
        bgx[32 * (h % 3):32 * (h % 3) + DH, h // 3] = bg[h * DH:(h + 1) * DH]
    woe = _expand_rows(Wo.astype(f32))

    maskf = mask_b[0].astype(f32)  # [R, W]
    x_flat = x.reshape(R, W, DN)
    in_maps2 = []
    for c in range(NC):
        mrows = maskf[c * RPC:(c + 1) * RPC]  # [RPC, W]
        ngj = (mrows.reshape(RPC, 2, 128) - 1.0) * 1e38  # [r, jt, p]
        ngj = np.ascontiguousarray(
            ngj.transpose(2, 0, 1).reshape(128, RPC * 2))
        in_maps2.append({
            "x": np.ascontiguousarray(
                x_flat[c * RPC:(c + 1) * RPC].reshape(RPC * W, DN)),
            "wq": gq, "wk": gk, "wv": gv, "wg": gg, "wo": woe,
            "bg": bgx, "bo": bo.reshape(1, DN).astype(f32),
            "bt": bt, "ngj": ngj.astype(f32),
            "invm": (1.0 - mrows).astype(np.uint8),
        })
    return nc2, in_maps2


def kernel(**inputs):
    nc2, in_maps2 = _prep(**inputs)
    res2 = bass_utils.run_bass_kernel_spmd(nc2, in_maps2,
                                           core_ids=list(range(NC)),
                                           trace=TRACE)
    if TRACE:
        print("attn kernel exec_time_ns:", res2.exec_time_ns)
    out = np.concatenate(
        [res2.results[c]["o"].reshape(RPC, W, DN) for c in range(NC)],
        axis=0)
    return out.reshape(B, R, W, DN).astype(np.float32)
